# revision 4
# baseline (speedup 1.0000x reference)
"""Trainium2 Bass kernel for the ConvolutionalOverlap problem (fp16 pipeline).

Reference computation (x: [2, 1, 256, 256] f32, w1/w2 scalar):
    out[b, i, h, w] = w1 * x[b, 0, h, w - (i+1)//2] + w2 * x[b, 0, h, w + (i+2)//2]
    (terms outside [0, W) are zero), out shape [2, 256, 256, 256].

The correctness gate is rel_err < 2e-2 (normalized absolute max), so the
kernel computes and stores in fp16: the HBM write per core drops from
16 MB to 8 MB, halving the DMA-write roofline (~358 GB/s/NeuronCore).
End-to-end fp16 error is ~8e-4, 25x inside the gate.

Strategy (pure SPMD across 8 cores, identical program, different data):
  - Flatten (b, h) into 512 rows; shard 64 rows per core.
  - Partitions 0..63 compute output columns w in [0, 128) and hold x
    zero-padded by 128 on the left; partitions 64..127 compute w in
    [128, 256) and hold x unshifted.  One free-dim access pattern then
    serves all 128 partitions; zero padding implements boundary masks.
  - ACT stages A = w1*x and B = w2*x plus one-element-shifted copies
    Ao[j] = A[j+1], Bo[j] = B[j+1] (all fp16).
  - out[ch] = A[w' + 128 - s1(ch)] + B[w' + 128 + s2(ch)] with
    s1 = (ch+1)//2, s2 = (ch+2)//2.  DVE computes these as plain fp16
    tensor_tensor adds.  fp16 TT runs in 2x_1P mode (2 elem/cycle) only
    when every AP has innermost step +-1 and 4-byte-aligned run starts,
    so channels are split per group into 4 instruction classes
    (a, p) in {0,1}^2 with ch = c0 + 4t + 2a + p: the channel-dim
    stride becomes +-2 fp16 elements (4B) and each class picks A vs Ao
    (B vs Bo) so its start offset is even.
  - 6 channel groups; each group's out-DMA (alternating SP/ACT HWDGE
    rings) launches as soon as its 4 TT instructions finish.  The input
    DMA runs on the GPSIMD (SWDGE) queue so it never queues behind the
    big output transfers.

Per core: ~0.2 MB in, 8.39 MB out -> DMA-write-bound at ~23.4 us.
DVE busy/iteration ~18.5 us (16384 cycles of 2x TT + overheads), so the
steady-state loop is memory-bound at the fp16 write roofline.

The module exposes two builders sharing one body emitter:
  build_nc()       - single-shot kernel (used for grading/correctness)
  build_loop_nc()  - same body inside per-engine hardware loops; the
                     iteration count K is a runtime input tensor, so one
                     NEFF serves all K.  Used by bench_loop.py: per-iter
                     device time = (t(K_hi) - t(K_lo)) / (K_hi - K_lo).
Cross-iteration sync uses per-(engine, sem) cumulative-target registers
(wait_ge(sem, reg)), single-buffered: iteration i's producers wait on
iteration i-1's consumers, which costs nothing in steady state.
"""

import sys

import numpy as np

if "/opt/trn_rl_repo" not in sys.path:
    sys.path.insert(0, "/opt/trn_rl_repo")

import concourse.bass as bass
import concourse.mybir as mybir
from concourse.ap import AP

F16 = mybir.dt.float16
F32 = mybir.dt.float32
I32 = mybir.dt.int32
COPY = mybir.ActivationFunctionType.Copy
ADD = mybir.AluOpType.add

P = 128          # SBUF partitions
W = 256          # spatial width == number of output channels
WH = W // 2      # output columns per partition half
XW = 392         # padded x width (j in [0, 384) used, padded to 392)
ROWS = 512       # B * H
NCORES = 8
RPC = ROWS // NCORES  # rows per core (64)
# Channel group sizes (sum 256), each a multiple of 4.  Graduated so the
# first output DMA launches early; rings SP (g0,g2,g4) and ACT (g1,g3,g5)
# each carry 128 channels.
GROUPS = [16, 48, 48, 48, 64, 32]

_nc_cache = {}


def _sub(tile_ap, off, dims):
    """AP over `tile_ap`'s tensor: all 128 partitions, custom free dims."""
    if not isinstance(tile_ap, AP):
        tile_ap = tile_ap[:]
    part = list(tile_ap.ap)[0]
    return AP(
        tile_ap.tensor,
        tile_ap.offset + off,
        [list(part)] + [list(d) for d in dims],
    )


class _Tensors:
    pass


def _alloc(nc, ctx, with_k):
    t = _Tensors()
    t.xdram = nc.dram_tensor("xp", [P, XW], F16, kind="ExternalInput")
    t.wdram = nc.dram_tensor("wv", [P, 2], F32, kind="ExternalInput")
    if with_k:
        t.kdram = nc.dram_tensor("kk", [1, 1], I32, kind="ExternalInput")
    t.odram = nc.dram_tensor("out", [P, W * WH], F16, kind="ExternalOutput")

    t.Xp = ctx.enter_context(nc.sbuf_tensor("Xp", [P, XW], F16))
    t.Av = ctx.enter_context(nc.sbuf_tensor("Av", [P, XW], F16))
    t.Ao = ctx.enter_context(nc.sbuf_tensor("Ao", [P, XW], F16))
    t.Bv = ctx.enter_context(nc.sbuf_tensor("Bv", [P, XW], F16))
    t.Bo = ctx.enter_context(nc.sbuf_tensor("Bo", [P, XW], F16))
    t.Wv = ctx.enter_context(nc.sbuf_tensor("Wv", [P, 2], F32))
    if with_k:
        t.Ks = ctx.enter_context(nc.sbuf_tensor("Ks", [1, 1], I32))
    t.Os = [
        ctx.enter_context(nc.sbuf_tensor(f"O{g}", [P, n * WH], F16))
        for g, n in enumerate(GROUPS)
    ]
    t.in_sem = ctx.enter_context(nc.semaphore("in_sem"))
    t.w_sem = ctx.enter_context(nc.semaphore("w_sem"))
    t.sab = ctx.enter_context(nc.semaphore("sab"))
    t.dg = [
        ctx.enter_context(nc.semaphore(f"dg{g}")) for g in range(len(GROUPS))
    ]
    t.mg = [
        ctx.enter_context(nc.semaphore(f"mg{g}")) for g in range(len(GROUPS))
    ]
    return t


def _emit_group(nc, t, O, c0, n):
    """4 fp16 TT instructions covering channels [c0, c0+n); returns last."""
    T = n // 4
    last = None
    for a in (0, 1):
        for p in (0, 1):
            q = c0 // 2 + a
            o1 = 128 - q - p           # A-side start (stride -2 per t)
            o2 = 129 + q               # B-side start (stride +2 per t)
            srcA, offA = (t.Av, o1) if o1 % 2 == 0 else (t.Ao, o1 - 1)
            srcB, offB = (t.Bv, o2) if o2 % 2 == 0 else (t.Bo, o2 - 1)
            in0 = _sub(srcA, offA, [(-2, T), (1, WH)])
            in1 = _sub(srcB, offB, [(2, T), (1, WH)])
            o = _sub(O, (2 * a + p) * WH, [(4 * WH, T), (1, WH)])
            last = nc.vector.tensor_tensor(o, in0, in1, ADD)
    return last


def _emit_iter(nc, t, wait):
    """One full iteration.  wait(eng, sem, c, lag, D) emits a wait whose
    target at iteration i is c + D*(i - lag)."""
    # GPSIMD/SWDGE: input DMA (off the HWDGE rings).  Xp may be
    # overwritten once ACT's previous-iteration staging is done.
    wait(nc.gpsimd, t.sab, 1, 1, 1)
    nc.gpsimd.dma_start(out=t.Xp[:], in_=t.xdram[:]).then_inc(t.in_sem, 16)

    # ACT: stage A/Ao/B/Bo.  Needs this iteration's input and DVE done
    # reading the staging tensors from the previous iteration.
    wait(nc.scalar, t.in_sem, 16, 0, 16)
    wait(nc.scalar, t.dg[-1], 1, 1, 1)
    W1 = t.Wv[:, 0:1]
    W2 = t.Wv[:, 1:2]
    nc.scalar.activation(t.Av[:, 0:XW], t.Xp[:, 0:XW], COPY, 0.0, W1)
    nc.scalar.activation(t.Ao[:, 0:256], t.Xp[:, 1:257], COPY, 0.0, W1)
    nc.scalar.activation(t.Bv[:, 0:XW], t.Xp[:, 0:XW], COPY, 0.0, W2)
    nc.scalar.activation(
        t.Bo[:, 0:384], t.Xp[:, 1:385], COPY, 0.0, W2
    ).then_inc(t.sab, 1)

    # DVE: 4 TT instructions per channel group.  Before writing a group's
    # O buffer, its previous-iteration out-DMA must have drained.
    wait(nc.vector, t.sab, 1, 0, 1)
    c0 = 0
    for g, n in enumerate(GROUPS):
        wait(nc.vector, t.mg[g], 16, 1, 16)
        _emit_group(nc, t, t.Os[g], c0, n).then_inc(t.dg[g], 1)
        c0 += n

    # Out-DMAs alternate between the two HWDGE rings (SP / ACT).
    c0 = 0
    for g, n in enumerate(GROUPS):
        eng = nc.sync if g % 2 == 0 else nc.scalar
        wait(eng, t.dg[g], 1, 0, 1)
        eng.dma_start(
            out=t.odram[:, c0 * WH:(c0 + n) * WH], in_=t.Os[g][:]
        ).then_inc(t.mg[g], 16)
        c0 += n


def _iter0_waiter(nc, t):
    """Immediate waits for the peeled iteration 0 (skip lag-satisfied)."""

    def wait(eng, sem, c, lag, D):
        if c - lag * D > 0:
            eng.wait_ge(sem, c)

    return wait


class _LoopWaiter:
    """Per-(engine, sem) cumulative wait targets held in registers.

    Register value entering iteration i equals last_const + D*(i-1) (the
    state after iteration i-1); each wait point advances it by a static
    delta and waits.  Initialized to the iteration-0 exit state."""

    def __init__(self, nc):
        self.nc = nc
        self.state = {}

    def _key(self, eng, sem):
        return (eng.engine, sem.name)

    def prepare(self, eng, sem, last_const):
        key = self._key(eng, sem)
        reg = eng.alloc_register(f"w_{sem.name}_{eng.engine.name}")
        eng.reg_mov(reg, last_const)
        self.state[key] = {"reg": reg, "const": last_const, "fresh": True}

    def begin_iteration(self):
        for st in self.state.values():
            st["fresh"] = True

    def wait(self, eng, sem, c, lag, D):
        st = self.state[self._key(eng, sem)]
        tconst = c - lag * D
        delta = tconst - st["const"]
        if st["fresh"]:
            delta += D
            st["fresh"] = False
        if delta:
            eng.reg_add(st["reg"], st["reg"], delta)
        eng.wait_ge(sem, st["reg"])
        st["const"] = tconst

    def reg(self, eng, sem):
        return self.state[self._key(eng, sem)]["reg"]


# Static wait-point table: (engine_attr, sem_attr, [(c, lag, D), ...]).
def _wait_points():
    pts = []
    ng = len(GROUPS)
    pts.append(("gpsimd", ("sab", None), [(1, 1, 1)]))
    pts.append(("scalar", ("in_sem", None), [(16, 0, 16)]))
    # ACT waits dg[last] twice: staging guard (lag 1) + g5 out-DMA (lag 0)
    pts.append(("scalar", ("dg", ng - 1), [(1, 1, 1), (1, 0, 1)]))
    pts.append(("vector", ("sab", None), [(1, 0, 1)]))
    for g in range(ng):
        pts.append(("vector", ("mg", g), [(16, 1, 16)]))
    for g in range(ng):
        eng = "sync" if g % 2 == 0 else "scalar"
        if not (eng == "scalar" and g == ng - 1):
            pts.append((eng, ("dg", g), [(1, 0, 1)]))
    return pts


def _sem_of(t, spec):
    name, idx = spec
    v = getattr(t, name)
    return v[idx] if idx is not None else v


def _preamble(nc, t):
    """Load Wv once (callers add the matching waits)."""
    nc.sync.dma_start(out=t.Wv[:], in_=t.wdram[:]).then_inc(t.w_sem, 16)


def build_nc():
    """Single-shot kernel: one iteration, drain, done."""
    from contextlib import ExitStack

    nc = bass.Bass(trn_type="TRN2")
    with ExitStack() as ctx:
        t = _alloc(nc, ctx, with_k=False)
        _preamble(nc, t)
        nc.scalar.wait_ge(t.w_sem, 16)
        _emit_iter(nc, t, _iter0_waiter(nc, t))
        for g in range(len(GROUPS)):
            eng = nc.sync if g % 2 == 0 else nc.scalar
            eng.wait_ge(t.mg[g], 16)
    return nc


def build_loop_nc():
    """Looped kernel for steady-state benching; K read from input `kk`.

    Do-while loop body runs i = 1..K-1 plus the peeled iteration 0, so a
    K value of n >= 2 executes exactly n iterations."""
    from contextlib import ExitStack

    ENG = [
        mybir.EngineType.SP,
        mybir.EngineType.Activation,
        mybir.EngineType.DVE,
        mybir.EngineType.Pool,
    ]
    nc = bass.Bass(trn_type="TRN2")
    with ExitStack() as ctx:
        t = _alloc(nc, ctx, with_k=True)
        _preamble(nc, t)
        # K to registers on every looping engine.
        nc.sync.dma_start(out=t.Ks[:], in_=t.kdram[:]).then_inc(t.w_sem, 16)
        for eng in (nc.sync, nc.scalar, nc.vector, nc.gpsimd):
            eng.wait_ge(t.w_sem, 32)
        kval = nc.values_load(
            t.Ks[0:1, 0:1], engines=ENG, min_val=2, max_val=1 << 30,
            skip_runtime_bounds_check=True,
        )

        # Peeled iteration 0.
        _emit_iter(nc, t, _iter0_waiter(nc, t))

        # Register waits initialized to the iteration-0 exit state.
        lw = _LoopWaiter(nc)
        engs = {"gpsimd": nc.gpsimd, "scalar": nc.scalar,
                "vector": nc.vector, "sync": nc.sync}
        for ename, sspec, pts in _wait_points():
            c, lag, D = pts[-1]
            lw.prepare(engs[ename], _sem_of(t, sspec), c - lag * D)

        with nc.Fori(1, kval, engines=ENG):
            lw.begin_iteration()
            _emit_iter(nc, t, lw.wait)

        # Drain: all out-DMAs of the final iteration complete.
        for eng_nc, eng_ty in ((nc.sync, "sync"), (nc.scalar, "scalar")):
            dreg = eng_nc.alloc_register(f"drain_{eng_ty}")
            eng_nc.reg_alu(dreg, kval, 16, mybir.AluOpType.mult)
            for g in range(len(GROUPS)):
                if (g % 2 == 0) == (eng_ty == "sync"):
                    eng_nc.wait_ge(t.mg[g], dreg)
    return nc


def get_nc(kind="single"):
    if kind not in _nc_cache:
        _nc_cache[kind] = build_nc() if kind == "single" else build_loop_nc()
    return _nc_cache[kind]


def prep_in_maps(x, w1, w2, kiters=None):
    """Shard + stage inputs for the 8 cores (host-side data movement only)."""
    x2 = np.ascontiguousarray(np.asarray(x, dtype=np.float32)[:, 0]).reshape(
        ROWS, W
    )
    xh = x2.astype(np.float16)
    w1v = np.float32(np.asarray(w1).reshape(-1)[0])
    w2v = np.float32(np.asarray(w2).reshape(-1)[0])
    in_maps = []
    for c in range(NCORES):
        rows = xh[c * RPC:(c + 1) * RPC]  # [64, 256]
        xp = np.zeros((P, XW), dtype=np.float16)
        xp[:RPC, 128:128 + W] = rows      # half 0: columns w in [0, 128)
        xp[RPC:, 0:W] = rows              # half 1: columns w in [128, 256)
        wv = np.empty((P, 2), dtype=np.float32)
        wv[:, 0] = w1v
        wv[:, 1] = w2v
        m = {"xp": xp, "wv": wv}
        if kiters is not None:
            m["kk"] = np.full((1, 1), int(kiters), dtype=np.int32)
        in_maps.append(m)
    return in_maps


def gather(outs):
    """Reassemble per-core [128, 256*128] f16 outputs into [2,256,256,256] f32."""
    parts = []
    for oc in outs:
        oc = np.asarray(oc).reshape(2, RPC, W, WH)  # [whalf, row, ch, w']
        parts.append(oc.transpose(1, 2, 0, 3).reshape(RPC, W, W))
    out_rows = np.concatenate(parts, axis=0)        # [512 rows, ch, w]
    return np.ascontiguousarray(
        out_rows.reshape(2, 256, W, W).transpose(0, 2, 1, 3)
    ).astype(np.float32)


def kernel(x, w1, w2, _run_kwargs=None):
    from concourse.bass_utils import run_bass_kernel_spmd

    nc = get_nc("single")
    in_maps = prep_in_maps(x, w1, w2)
    kwargs = _run_kwargs or {}
    res = run_bass_kernel_spmd(nc, in_maps, core_ids=list(range(NCORES)), **kwargs)
    out = gather([r["out"] for r in res.results])
    if kwargs:
        kernel.last_results = res
    return out


# revision 21
# speedup vs baseline: 1.0627x; 1.0627x over previous
"""Trainium2 Bass kernel for the ConvolutionalOverlap problem (fp16 pipeline).

Reference computation (x: [2, 1, 256, 256] f32, w1/w2 scalar):
    out[b, i, h, w] = w1 * x[b, 0, h, w - (i+1)//2] + w2 * x[b, 0, h, w + (i+2)//2]
    (terms outside [0, W) are zero), out shape [2, 256, 256, 256].

The correctness gate is rel_err < 2e-2 (normalized absolute max), so the
kernel computes and stores in fp16: the HBM write per core drops from
16 MB to 8 MB, halving the DMA-write roofline (~358 GB/s/NeuronCore).
End-to-end fp16 error is ~8e-4, 25x inside the gate.

Strategy (pure SPMD across 8 cores, identical program, different data):
  - Flatten (b, h) into 512 rows; shard 64 rows per core.
  - Partitions 0..63 compute output columns w in [0, 128) and hold x
    zero-padded by 128 on the left; partitions 64..127 compute w in
    [128, 256) and hold x unshifted.  One free-dim access pattern then
    serves all 128 partitions; zero padding implements boundary masks.
  - ACT stages A = w1*x and B = w2*x plus one-element-shifted copies
    Ao[j] = A[j+1], Bo[j] = B[j+1] (all fp16).
  - out[ch] = A[w' + 128 - s1(ch)] + B[w' + 128 + s2(ch)] with
    s1 = (ch+1)//2, s2 = (ch+2)//2.  DVE computes these as plain fp16
    tensor_tensor adds.  fp16 TT runs in 2x_1P mode (2 elem/cycle) only
    when every AP has innermost step +-1 and 4-byte-aligned run starts,
    so channels are split per group into 4 instruction classes
    (a, p) in {0,1}^2 with ch = c0 + 4t + 2a + p: the channel-dim
    stride becomes +-2 fp16 elements (4B) and each class picks A vs Ao
    (B vs Bo) so its start offset is even.
  - 2 channel groups of 128, one out-DMA per HWDGE ring (SP / ACT);
    each launches as soon as its 4 TT instructions finish.  Each ring
    sustains only ~183 GB/s and the NC total caps at ~366 GB/s, so the
    rings must carry equal bytes and fewer/bigger DMAs win (HW A/B:
    2 groups 23.2us, 4 groups 23.8us, 6 groups 24.7us; 192/64 ring
    imbalance 32.8us; routing a 3rd stream through the SWDGE queue
    gains nothing - the per-NC cap binds).  The input DMA runs on the
    GPSIMD (SWDGE) queue so it never queues behind the output rings.

Per core: ~0.1 MB in, 8.39 MB out -> DMA-bound at ~23.2 us/iteration
steady state (measured; 2.1x the f32 baseline's 49.2 us).  DVE
busy/iteration ~17.5 us (16384 cycles of 2x TT + overheads), under the
DMA bound, so the loop is memory-bound at the fp16 write roofline.

The module exposes two builders sharing one body emitter:
  build_nc()       - single-shot kernel (used for grading/correctness)
  build_loop_nc()  - same body inside per-engine hardware loops; the
                     iteration count K is a runtime input tensor, so one
                     NEFF serves all K.  Used by bench_loop.py: per-iter
                     device time = (t(K_hi) - t(K_lo)) / (K_hi - K_lo).
Cross-iteration sync uses per-(engine, sem) cumulative-target registers
(wait_ge(sem, reg)), single-buffered: iteration i's producers wait on
iteration i-1's consumers, which costs nothing in steady state.
"""

import sys

import numpy as np

if "/opt/trn_rl_repo" not in sys.path:
    sys.path.insert(0, "/opt/trn_rl_repo")

import concourse.bass as bass
import concourse.mybir as mybir
from concourse.ap import AP

F16 = mybir.dt.float16
F32 = mybir.dt.float32
I32 = mybir.dt.int32
COPY = mybir.ActivationFunctionType.Copy
ADD = mybir.AluOpType.add

P = 128          # SBUF partitions
W = 256          # spatial width == number of output channels
WH = W // 2      # output columns per partition half
XW = 392         # padded x width (j in [0, 384) used, padded to 392)
ROWS = 512       # B * H
NCORES = 8
RPC = ROWS // NCORES  # rows per core (64)
# Channel group sizes (sum 256), each a multiple of 4.  One group per
# HWDGE ring: each ring sustains only ~183 GB/s, so the two rings must
# carry equal bytes, and fewer/bigger DMAs measurably beat many small
# ones (HW A/B: 2 groups 23.2us, 4 groups 23.8us, 6 groups 24.7us,
# 192/64 imbalance 32.8us).
GROUPS = [128, 128]
# If True, the per-iteration input DMA is issued on the SP HWDGE ring
# (first in ring order) instead of the GPSIMD/SWDGE queue.  Only legal
# when total HWDGE DMA instructions stay within the 8 DMAHW sem lanes.
IN_ON_SP = False
# Out-DMA queue per group: "sp"/"act" are the two HWDGE rings, "gp" is
# the SWDGE (GPSIMD) queue — a third, independent descriptor stream.
# None -> alternate sp/act.  Each HWDGE ring sustains ~183 GB/s, so
# balance bytes across the streams.
RINGS = None


def _ring_of(g):
    if RINGS is not None:
        return RINGS[g]
    return "sp" if g % 2 == 0 else "act"

_nc_cache = {}


def _sub(tile_ap, off, dims):
    """AP over `tile_ap`'s tensor: all 128 partitions, custom free dims."""
    if not isinstance(tile_ap, AP):
        tile_ap = tile_ap[:]
    part = list(tile_ap.ap)[0]
    return AP(
        tile_ap.tensor,
        tile_ap.offset + off,
        [list(part)] + [list(d) for d in dims],
    )


class _Tensors:
    pass


def _alloc(nc, ctx, with_k):
    t = _Tensors()
    t.xdram = nc.dram_tensor("xp", [P, XW], F16, kind="ExternalInput")
    t.wdram = nc.dram_tensor("wv", [P, 2], F32, kind="ExternalInput")
    if with_k:
        t.kdram = nc.dram_tensor("kk", [1, 1], I32, kind="ExternalInput")
    t.odram = nc.dram_tensor("out", [P, W * WH], F16, kind="ExternalOutput")

    t.Xp = ctx.enter_context(nc.sbuf_tensor("Xp", [P, XW], F16))
    t.Av = ctx.enter_context(nc.sbuf_tensor("Av", [P, XW], F16))
    t.Ao = ctx.enter_context(nc.sbuf_tensor("Ao", [P, XW], F16))
    t.Bv = ctx.enter_context(nc.sbuf_tensor("Bv", [P, XW], F16))
    t.Bo = ctx.enter_context(nc.sbuf_tensor("Bo", [P, XW], F16))
    t.Wv = ctx.enter_context(nc.sbuf_tensor("Wv", [P, 2], F32))
    if with_k:
        t.Ks = ctx.enter_context(nc.sbuf_tensor("Ks", [1, 1], I32))
    t.Os = [
        ctx.enter_context(nc.sbuf_tensor(f"O{g}", [P, n * WH], F16))
        for g, n in enumerate(GROUPS)
    ]
    t.in_sem = ctx.enter_context(nc.semaphore("in_sem"))
    t.w_sem = ctx.enter_context(nc.semaphore("w_sem"))
    t.sab = ctx.enter_context(nc.semaphore("sab"))
    t.dg = [
        ctx.enter_context(nc.semaphore(f"dg{g}")) for g in range(len(GROUPS))
    ]
    t.mg = [
        ctx.enter_context(nc.semaphore(f"mg{g}")) for g in range(len(GROUPS))
    ]
    return t


def _emit_group(nc, t, O, c0, n):
    """4 fp16 TT instructions covering channels [c0, c0+n); returns last."""
    T = n // 4
    last = None
    for a in (0, 1):
        for p in (0, 1):
            q = c0 // 2 + a
            o1 = 128 - q - p           # A-side start (stride -2 per t)
            o2 = 129 + q               # B-side start (stride +2 per t)
            srcA, offA = (t.Av, o1) if o1 % 2 == 0 else (t.Ao, o1 - 1)
            srcB, offB = (t.Bv, o2) if o2 % 2 == 0 else (t.Bo, o2 - 1)
            in0 = _sub(srcA, offA, [(-2, T), (1, WH)])
            in1 = _sub(srcB, offB, [(2, T), (1, WH)])
            o = _sub(O, (2 * a + p) * WH, [(4 * WH, T), (1, WH)])
            last = nc.vector.tensor_tensor(o, in0, in1, ADD)
    return last


def _emit_iter(nc, t, wait):
    """One full iteration.  wait(eng, sem, c, lag, D) emits a wait whose
    target at iteration i is c + D*(i - lag)."""
    # Input DMA.  Xp may be overwritten once ACT's previous-iteration
    # staging is done.  SWDGE (GPSIMD) keeps it off the HWDGE rings;
    # IN_ON_SP instead puts it first in SP ring order.
    in_eng = nc.sync if IN_ON_SP else nc.gpsimd
    wait(in_eng, t.sab, 1, 1, 1)
    in_eng.dma_start(out=t.Xp[:], in_=t.xdram[:]).then_inc(t.in_sem, 16)

    # ACT: stage A/Ao/B/Bo.  Needs this iteration's input and DVE done
    # reading the staging tensors from the previous iteration.
    wait(nc.scalar, t.in_sem, 16, 0, 16)
    wait(nc.scalar, t.dg[-1], 1, 1, 1)
    W1 = t.Wv[:, 0:1]
    W2 = t.Wv[:, 1:2]
    nc.scalar.activation(t.Av[:, 0:XW], t.Xp[:, 0:XW], COPY, 0.0, W1)
    nc.scalar.activation(t.Ao[:, 0:256], t.Xp[:, 1:257], COPY, 0.0, W1)
    nc.scalar.activation(t.Bv[:, 0:XW], t.Xp[:, 0:XW], COPY, 0.0, W2)
    nc.scalar.activation(
        t.Bo[:, 0:384], t.Xp[:, 1:385], COPY, 0.0, W2
    ).then_inc(t.sab, 1)

    # DVE: 4 TT instructions per channel group.  Before writing a group's
    # O buffer, its previous-iteration out-DMA must have drained.
    wait(nc.vector, t.sab, 1, 0, 1)
    c0 = 0
    for g, n in enumerate(GROUPS):
        wait(nc.vector, t.mg[g], 16, 1, 16)
        _emit_group(nc, t, t.Os[g], c0, n).then_inc(t.dg[g], 1)
        c0 += n

    # Out-DMAs spread across the two HWDGE rings (SP / ACT) and
    # optionally the SWDGE queue.
    engs = {"sp": nc.sync, "act": nc.scalar, "gp": nc.gpsimd}
    c0 = 0
    for g, n in enumerate(GROUPS):
        eng = engs[_ring_of(g)]
        wait(eng, t.dg[g], 1, 0, 1)
        eng.dma_start(
            out=t.odram[:, c0 * WH:(c0 + n) * WH], in_=t.Os[g][:]
        ).then_inc(t.mg[g], 16)
        c0 += n


def _iter0_waiter(nc, t):
    """Immediate waits for the peeled iteration 0 (skip lag-satisfied)."""

    def wait(eng, sem, c, lag, D):
        if c - lag * D > 0:
            eng.wait_ge(sem, c)

    return wait


class _LoopWaiter:
    """Per-(engine, sem) cumulative wait targets held in registers.

    Register value entering iteration i equals last_const + D*(i-1) (the
    state after iteration i-1); each wait point advances it by a static
    delta and waits.  Initialized to the iteration-0 exit state."""

    def __init__(self, nc):
        self.nc = nc
        self.state = {}

    def _key(self, eng, sem):
        return (eng.engine, sem.name)

    def prepare(self, eng, sem, last_const):
        key = self._key(eng, sem)
        reg = eng.alloc_register(f"w_{sem.name}_{eng.engine.name}")
        eng.reg_mov(reg, last_const)
        self.state[key] = {"reg": reg, "const": last_const, "fresh": True}

    def begin_iteration(self):
        for st in self.state.values():
            st["fresh"] = True

    def wait(self, eng, sem, c, lag, D):
        st = self.state[self._key(eng, sem)]
        tconst = c - lag * D
        delta = tconst - st["const"]
        if st["fresh"]:
            delta += D
            st["fresh"] = False
        if delta:
            eng.reg_add(st["reg"], st["reg"], delta)
        eng.wait_ge(sem, st["reg"])
        st["const"] = tconst

    def reg(self, eng, sem):
        return self.state[self._key(eng, sem)]["reg"]


# Wait-point table mirroring _emit_iter's wait() calls, in emission order.
# Maps (engine, sem) -> the LAST wait's (c - lag*D), i.e. the iteration-0
# exit state each cumulative register must be initialized to.
def _wait_points():
    pts = {}
    ng = len(GROUPS)
    pts[("sync" if IN_ON_SP else "gpsimd", ("sab", None))] = 1 - 1
    pts[("scalar", ("in_sem", None))] = 16
    pts[("scalar", ("dg", ng - 1))] = 1 - 1
    pts[("vector", ("sab", None))] = 1
    for g in range(ng):
        pts[("vector", ("mg", g))] = 16 - 16
    ename = {"sp": "sync", "act": "scalar", "gp": "gpsimd"}
    for g in range(ng):
        pts[(ename[_ring_of(g)], ("dg", g))] = 1
    return pts


def _sem_of(t, spec):
    name, idx = spec
    v = getattr(t, name)
    return v[idx] if idx is not None else v


def _preamble(nc, t):
    """Load Wv once (callers add the matching waits)."""
    nc.sync.dma_start(out=t.Wv[:], in_=t.wdram[:]).then_inc(t.w_sem, 16)


def build_nc():
    """Single-shot kernel: one iteration, drain, done."""
    from contextlib import ExitStack

    nc = bass.Bass(trn_type="TRN2")
    with ExitStack() as ctx:
        t = _alloc(nc, ctx, with_k=False)
        _preamble(nc, t)
        nc.scalar.wait_ge(t.w_sem, 16)
        _emit_iter(nc, t, _iter0_waiter(nc, t))
        engs = {"sp": nc.sync, "act": nc.scalar, "gp": nc.gpsimd}
        for g in range(len(GROUPS)):
            engs[_ring_of(g)].wait_ge(t.mg[g], 16)
    return nc


def build_loop_nc():
    """Looped kernel for steady-state benching; K read from input `kk`.

    Do-while loop body runs i = 1..K-1 plus the peeled iteration 0, so a
    K value of n >= 2 executes exactly n iterations."""
    from contextlib import ExitStack

    use_pool = (not IN_ON_SP) or any(
        _ring_of(g) == "gp" for g in range(len(GROUPS))
    )
    ENG = [
        mybir.EngineType.SP,
        mybir.EngineType.Activation,
        mybir.EngineType.DVE,
    ]
    if use_pool:
        ENG.append(mybir.EngineType.Pool)
    nc = bass.Bass(trn_type="TRN2")
    with ExitStack() as ctx:
        t = _alloc(nc, ctx, with_k=True)
        _preamble(nc, t)
        # K to registers on every looping engine.
        nc.sync.dma_start(out=t.Ks[:], in_=t.kdram[:]).then_inc(t.w_sem, 16)
        loop_engs = [nc.sync, nc.scalar, nc.vector]
        if use_pool:
            loop_engs.append(nc.gpsimd)
        for eng in loop_engs:
            eng.wait_ge(t.w_sem, 32)
        kval = nc.values_load(
            t.Ks[0:1, 0:1], engines=ENG, min_val=2, max_val=1 << 30,
            skip_runtime_bounds_check=True,
        )

        # Peeled iteration 0.
        _emit_iter(nc, t, _iter0_waiter(nc, t))

        # Register waits initialized to the iteration-0 exit state.
        lw = _LoopWaiter(nc)
        engs = {"gpsimd": nc.gpsimd, "scalar": nc.scalar,
                "vector": nc.vector, "sync": nc.sync}
        for (ename, sspec), last_const in _wait_points().items():
            lw.prepare(engs[ename], _sem_of(t, sspec), last_const)

        with nc.Fori(1, kval, engines=ENG):
            lw.begin_iteration()
            _emit_iter(nc, t, lw.wait)

        # Drain: all out-DMAs of the final iteration complete.
        engs = {"sp": nc.sync, "act": nc.scalar, "gp": nc.gpsimd}
        for ring, eng_nc in engs.items():
            gs = [g for g in range(len(GROUPS)) if _ring_of(g) == ring]
            if not gs:
                continue
            dreg = eng_nc.alloc_register(f"drain_{ring}")
            eng_nc.reg_alu(dreg, kval, 16, mybir.AluOpType.mult)
            for g in gs:
                eng_nc.wait_ge(t.mg[g], dreg)
    return nc


def get_nc(kind="single"):
    if kind not in _nc_cache:
        _nc_cache[kind] = build_nc() if kind == "single" else build_loop_nc()
    return _nc_cache[kind]


def prep_in_maps(x, w1, w2, kiters=None):
    """Shard + stage inputs for the 8 cores (host-side data movement only)."""
    x2 = np.ascontiguousarray(np.asarray(x, dtype=np.float32)[:, 0]).reshape(
        ROWS, W
    )
    xh = x2.astype(np.float16)
    w1v = np.float32(np.asarray(w1).reshape(-1)[0])
    w2v = np.float32(np.asarray(w2).reshape(-1)[0])
    in_maps = []
    for c in range(NCORES):
        rows = xh[c * RPC:(c + 1) * RPC]  # [64, 256]
        xp = np.zeros((P, XW), dtype=np.float16)
        xp[:RPC, 128:128 + W] = rows      # half 0: columns w in [0, 128)
        xp[RPC:, 0:W] = rows              # half 1: columns w in [128, 256)
        wv = np.empty((P, 2), dtype=np.float32)
        wv[:, 0] = w1v
        wv[:, 1] = w2v
        m = {"xp": xp, "wv": wv}
        if kiters is not None:
            m["kk"] = np.full((1, 1), int(kiters), dtype=np.int32)
        in_maps.append(m)
    return in_maps


def gather(outs):
    """Reassemble per-core [128, 256*128] f16 outputs into [2,256,256,256] f32."""
    parts = []
    for oc in outs:
        oc = np.asarray(oc).reshape(2, RPC, W, WH)  # [whalf, row, ch, w']
        parts.append(oc.transpose(1, 2, 0, 3).reshape(RPC, W, W))
    out_rows = np.concatenate(parts, axis=0)        # [512 rows, ch, w]
    return np.ascontiguousarray(
        out_rows.reshape(2, 256, W, W).transpose(0, 2, 1, 3)
    ).astype(np.float32)


def kernel(x, w1, w2, _run_kwargs=None):
    from concourse.bass_utils import run_bass_kernel_spmd

    nc = get_nc("single")
    in_maps = prep_in_maps(x, w1, w2)
    kwargs = _run_kwargs or {}
    res = run_bass_kernel_spmd(nc, in_maps, core_ids=list(range(NCORES)), **kwargs)
    out = gather([r["out"] for r in res.results])
    if kwargs:
        kernel.last_results = res
    return out
